# revision 40
# baseline (speedup 1.0000x reference)
"""Multi-head attention block on 8 Trainium2 NeuronCores.

Sharding: batch (B=2) x head-groups (4 heads each) -> 8 cores.
Each core computes q/k/v projections for its 4 heads of its batch,
causal attention, and a partial output projection; the host sums the
4 partials per batch and adds the bias.

Layout:
  qT/kT  = W @ x^T            [d_k*4, s]  (x^T supplied pre-transposed)
  S^T    = kT-slices^T @ qT   scores block [k, q] per head; the causal
                              diagonal gets an additive -1e9 triangle via
                              an identity-lhsT matmul into the same psum
  E      = exp(S^T)           bf16, directly consumable by PV
  A      = (E^T V_aug)        FLIPPED PV: out [q, 65] per (q-chunk,head),
                              all 4 q-chunks packed in ONE psum bank
                              (has_written overwrite-where-unset);
                              column 64 (ones in V_aug) is the softmax
                              denominator; normalize with a per-partition
                              reciprocal (q is on partitions).
  A^T    via DMA transpose    [hd, q] chunks feed the out-projection,
                              which is deferred to the kernel tail as
                              PE filler for the exp-bound stretches.
Biases: b_q applied on-chip (per-partition). b_k is DROPPED: softmax is
invariant to adding (q+bq).bk (constant per query); the result is exact.
b_v and b_out fold into a host-side constant added after gathering.
"""

import os
import re
import sys
from contextlib import nullcontext

sys.path.insert(0, "/opt/trn_rl_repo")

import numpy as np
import ml_dtypes

import concourse.bass as bass
import concourse.mybir as mybir
import concourse.tile as tile
from concourse.masks import make_identity

BF16 = mybir.dt.bfloat16
F32 = mybir.dt.float32
BF16_NP = ml_dtypes.bfloat16

N_CORES = 8
B = 2
S = 2048
D_MODEL = 1024
H_TOTAL = 16
D_K = 64
H_PER_CORE = 4                      # heads per core
HD = H_PER_CORE * D_K               # 256 head-dims per core
CORES_PER_BATCH = N_CORES // B

QB = 512                            # q-block (matmul moving free dim)
KC = 128                            # k-chunk (contraction tile)


def _split_waits_json(bir_json: bytes, limit: int = 1) -> bytes:
    """walrus in this env rejects >limit sync-waits on an instruction.
    Hoist excess waits onto fresh NoOps inserted just before, on the same
    engine queue (queue execution is serial, so ordering is identical)."""
    import orjson

    m = orjson.loads(bir_json)
    ctr = 0
    for fn in m.get("functions", []):
        for bb in fn.get("blocks", []):
            insts = bb.get("instructions") or []
            if not any(
                len((i.get("sync_info") or {}).get("on_wait") or []) > limit
                for i in insts
            ):
                continue
            out = []
            for inst in insts:
                si = inst.get("sync_info")
                waits = (si or {}).get("on_wait") or []
                if len(waits) > limit:
                    for w in waits[:-limit]:
                        ctr += 1
                        out.append(
                            {
                                "debug": inst.get("debug", 0),
                                "engine": inst["engine"],
                                "ins": [],
                                "outs": [],
                                "name": f"WSPL-{ctr}",
                                "opcode": "NoOp",
                                "sync_info": {"on_update": [], "on_wait": [w]},
                            }
                        )
                    si["on_wait"] = waits[-limit:]
                out.append(inst)
            bb["instructions"] = out
    return orjson.dumps(m)


LAST_PREDICTED_NS = None


def _install_schedule_capture():
    """Record the Tile scheduler's cost-model makespan for each build."""
    if getattr(tile.TileContext, "_capture_installed", False):
        return
    orig = tile.TileContext.schedule_block

    def wrapped(self, *a, **kw):
        r = orig(self, *a, **kw)
        try:
            global LAST_PREDICTED_NS
            LAST_PREDICTED_NS = r[1].time
        except Exception:
            pass
        return r

    tile.TileContext.schedule_block = wrapped
    tile.TileContext._capture_installed = True


def _install_compile_patch():
    import concourse.bass_utils as bu
    import concourse.bass2jax as b2j

    if getattr(bu, "_wait_split_installed", False):
        return
    orig = bu.compile_bir_kernel

    def wrapped(bir_json, tmpdir, neff_name="file.neff"):
        return orig(_split_waits_json(bytes(bir_json)), tmpdir, neff_name)

    bu.compile_bir_kernel = wrapped
    b2j.compile_bir_kernel = wrapped
    bu._wait_split_installed = True


def build_program(mask_mode="causal", s=S, d=D_MODEL, heads=H_PER_CORE,
                  epool_bufs=24, a2pool_bufs=12, opool_bufs=3,
                  pp_bufs=2, sp_bufs=2, app_bufs=2, unified_wp=False,
                  warm_mms=13, mpool_bufs=16,
                  dma_tr=True, pool_mask=True, split0=False):
    """One SPMD program; per-core behavior differs only via inputs.

    mask_mode: "causal" (skip above-diagonal chunks, affine-select the
    diagonal ones), "ones" (no masking), "general" (multiplicative 0/1
    mask loaded from DRAM, pre-transposed host-side).
    """
    _install_compile_patch()
    _install_schedule_capture()
    hd = heads * D_K
    nq = s // QB          # q blocks
    nkc = s // KC         # k chunks
    dch = d // 128        # contraction chunks for projections
    npair = heads // 2    # head pairs (even head on partitions 0-63)
    nqc = QB // 128       # 128-wide q chunks per q block
    assert hd % 128 == 0 and hd // 128 == npair

    nc = bass.Bass()
    xq = nc.dram_tensor("xq", [d, s], BF16, kind="ExternalInput")
    xk = nc.dram_tensor("xk", [d, s], BF16, kind="ExternalInput")
    xv = nc.dram_tensor("xv", [d, s], BF16, kind="ExternalInput")
    # weights arrive pre-packed host-side so each partition's line is one
    # contiguous 4KB run: wq[p, c*hd+m] = w_q.T[c*128+p, m] etc.
    wq = nc.dram_tensor("wq", [128, dch * hd], BF16, kind="ExternalInput")
    wk = nc.dram_tensor("wk", [128, dch * hd], BF16, kind="ExternalInput")
    wv = nc.dram_tensor("wv", [128, dch * hd], BF16, kind="ExternalInput")
    wo = nc.dram_tensor("wo", [128, npair * d], BF16, kind="ExternalInput")
    bq = nc.dram_tensor("bq", [hd, 1], F32, kind="ExternalInput")
    if mask_mode == "general":
        m01 = nc.dram_tensor("m01", [s, s], BF16, kind="ExternalInput")
    out = nc.dram_tensor("out", [s, d], BF16, kind="ExternalOutput")

    xq_r = xq[:, :].rearrange("(c p) s -> p c s", p=128)
    xk_r = xk[:, :].rearrange("(c p) s -> p c s", p=128)
    xv_r = xv[:, :].rearrange("(c p) s -> p c s", p=128)
    wq_r = wq[:, :].rearrange("p (c2 c m) -> p c2 c m", c2=npair, m=128)
    wk_r = wk[:, :].rearrange("p (c2 c m) -> p c2 c m", c2=npair, m=128)
    wv_r = wv[:, :].rearrange("p (c2 c m) -> p c2 c m", c2=npair, m=128)
    wo_r = wo[:, :].rearrange("p (c e) -> p c e", e=d)

    with tile.TileContext(nc) as tc:
        with (
            tc.tile_pool(name="consts", bufs=1) as consts,
            tc.tile_pool(name="qkres", bufs=1) as qkres,
            tc.tile_pool(name="xpool", bufs=4) as xpool,
            tc.tile_pool(name="epool", bufs=epool_bufs) as epool,
            tc.tile_pool(name="a2pool", bufs=a2pool_bufs) as a2pool,
            tc.tile_pool(name="atpool", bufs=5) as atpool,
            tc.tile_pool(name="opool", bufs=opool_bufs) as opool,
            tc.tile_pool(name="rpool", bufs=4) as rpool,
            tc.tile_pool(name="mpool", bufs=mpool_bufs) as mpool,
            tc.tile_pool(name="pp", bufs=(6 if unified_wp else pp_bufs),
                         space="PSUM") as pp,
            tc.tile_pool(name="sp", bufs=sp_bufs, space="PSUM") as sp,
            (tc.tile_pool(name="app", bufs=app_bufs, space="PSUM")
             if not unified_wp else nullcontext(None)) as app_raw,
        ):
            app = pp if unified_wp else app_raw
            app_tag = "pp" if unified_wp else "app"
            # --- constants + x inputs, DMA-ordered by first use:
            # xk(qb0), wk (c2 halves), bq, xq(qb0), wq, xv(qb0), wv, wo,
            # then per-qb x.  Weights are packed host-side as
            # w[p, (c2, dc, mm)] so a c2-half is one contiguous 2KB run.
            xk_t = [xpool.tile([128, dch, QB], BF16, tag="xk", name=f"xk{i}") for i in range(nq)]
            xq_t = [xpool.tile([128, dch, QB], BF16, tag="xq", name=f"xq{i}") for i in range(nq)]
            xv_t = [xpool.tile([128, dch, QB], BF16, tag="xv", name=f"xv{i}") for i in range(nq)]

            wk_sb = consts.tile([128, npair, dch, 128], BF16, tag="wk")
            wq_sb = consts.tile([128, npair, dch, 128], BF16, tag="wq")
            if split0:
                # qb0 x loads split by contraction halves: the first 4 chain
                # matmuls of each projection start on half the data (tile
                # range-deps gate per-matmul)
                nc.sync.dma_start(xk_t[0][:, 0:4, :], xk_r[:, 0:4, 0:QB])
                nc.sync.dma_start(wk_sb[:, 0], wk_r[:, 0])
                nc.sync.dma_start(xk_t[0][:, 4:8, :], xk_r[:, 4:8, 0:QB])
                nc.sync.dma_start(xq_t[0][:, 0:4, :], xq_r[:, 0:4, 0:QB])
                nc.sync.dma_start(wq_sb[:, 0], wq_r[:, 0])
                nc.sync.dma_start(xq_t[0][:, 4:8, :], xq_r[:, 4:8, 0:QB])
            else:
                nc.sync.dma_start(xk_t[0], xk_r[:, :, 0:QB])
                nc.sync.dma_start(wk_sb[:, 0], wk_r[:, 0])
                nc.sync.dma_start(xq_t[0], xq_r[:, :, 0:QB])
                nc.sync.dma_start(wq_sb[:, 0], wq_r[:, 0])
            bq_sb = consts.tile([128, npair], F32, tag="bq")
            for c2 in range(npair):
                nc.sync.dma_start(bq_sb[:, c2 : c2 + 1], bq[c2 * 128 : (c2 + 1) * 128, :])
            nc.sync.dma_start(wk_sb[:, 1], wk_r[:, 1])
            nc.sync.dma_start(wq_sb[:, 1], wq_r[:, 1])
            nc.sync.dma_start(xv_t[0], xv_r[:, :, 0:QB])
            wv_sb = consts.tile([128, npair, dch, 128], BF16, tag="wv")
            for c2 in range(npair):
                nc.sync.dma_start(wv_sb[:, c2], wv_r[:, c2])
            wo_sb = consts.tile([128, npair, d], BF16, tag="wo")
            for qb in range(1, nq):
                sl = slice(qb * QB, (qb + 1) * QB)
                nc.sync.dma_start(xk_t[qb], xk_r[:, :, sl])
                nc.sync.dma_start(xq_t[qb], xq_r[:, :, sl])
                nc.sync.dma_start(xv_t[qb], xv_r[:, :, sl])
                if qb == 2:
                    nc.sync.dma_start(wo_sb, wo_r)

            ident = consts.tile([128, 128], BF16, tag="ident")
            make_identity(nc, ident[:, :])
            # additive causal mask for the diagonal 128x128 of a score
            # chunk: -1e9 where query-col < key-partition, else 0
            ctri = consts.tile([128, 128], BF16, tag="ctri")
            nc.gpsimd.memset(ctri, 0.0)
            nc.gpsimd.affine_select(
                out=ctri, in_=ctri,
                compare_op=mybir.AluOpType.is_ge, fill=-1e9,
                base=0, pattern=[[1, 128]], channel_multiplier=-1,
            )
            if pool_mask:
                # multiplicative 0/1 triangle for post-exp diagonal masking
                tri01 = consts.tile([128, 128], BF16, tag="tri01")
                nc.gpsimd.memset(tri01, 1.0)
                nc.gpsimd.affine_select(
                    out=tri01, in_=tri01,
                    compare_op=mybir.AluOpType.is_ge, fill=0.0,
                    base=0, pattern=[[1, 128]], channel_multiplier=-1,
                )
            # preload the Exp activation table during the DMA ramp so the
            # first real exp doesn't pay ACT_TABLE_LOAD
            warm = consts.tile([1, 2], F32, tag="warm")
            nc.vector.memset(warm, 0.0)
            nc.scalar.activation(
                out=warm, in_=warm, func=mybir.ActivationFunctionType.Exp
            )
            # dummy matmuls spanning the initial DMA wait keep the PE p-state
            # ramp warm so the first real chains run at full clock
            if warm_mms:
                zsrc = consts.tile([128, QB], BF16, tag="zsrc")
                nc.vector.memset(zsrc, 0.0)
                wps = app.tile([128, QB], F32, tag=app_tag, name="wps")
                for _ in range(warm_mms):
                    nc.tensor.matmul(wps, lhsT=ident, rhs=zsrc,
                                     start=True, stop=True)

            # persistent per-core tensors
            qT = qkres.tile([128, npair, s], BF16, tag="qT")
            kT = qkres.tile([128, npair, s], BF16, tag="kT")
            v_sb = qkres.tile([128, nkc, heads, 65], BF16, tag="v")
            # ones column 64 of each head-block = softmax denominator trick
            nc.vector.memset(v_sb[:, :, :, 64:65], 1.0)

            def proj_block(qb):
                s_lo = qb * QB

                def kq_proj_group(x_t, w_sb, dst, c2, with_bias, lo=0, w=QB):
                    ps = pp.tile([128, QB], F32, tag="pp", name="ps")
                    with tc.high_priority():
                        for dc in range(dch):
                            nc.tensor.matmul(
                                ps[:, 0:w],
                                lhsT=w_sb[:, c2, dc, :],
                                rhs=x_t[:, dc, lo : lo + w],
                                start=(dc == 0),
                                stop=(dc == dch - 1),
                            )
                    with tc.high_priority():
                        if with_bias:
                            nc.vector.tensor_scalar_add(
                                dst[:, c2, s_lo + lo : s_lo + lo + w], ps[:, 0:w],
                                bq_sb[:, c2 : c2 + 1],
                            )
                        else:
                            nc.vector.tensor_copy(
                                dst[:, c2, s_lo + lo : s_lo + lo + w], ps[:, 0:w]
                            )

                def v_proj_group(sc, qb=None):
                    sck = qb * nqc + sc
                    ps = pp.tile([128, hd], F32, tag="pp", name="ps")
                    for dc in range(dch):
                        nc.tensor.matmul(
                            ps,
                            lhsT=xv_t[qb][:, dc, sc * 128 : (sc + 1) * 128],
                            rhs=wv_sb[:, :, dc, :],
                            start=(dc == 0),
                            stop=(dc == dch - 1),
                        )
                    nc.vector.tensor_copy(
                        v_sb[:, sck, :, 0:64],
                        ps[:].rearrange("p (h j) -> p h j", j=64),
                    )

                def v_block(qb=qb):
                    for sc in range(nqc):
                        v_proj_group(sc, qb)

                v_blocks[qb] = v_block

                # k/q interleaved per pair so pair-0 scores can start after
                # just two chains
                for c2 in range(npair):
                    kq_proj_group(xk_t[qb], wk_sb, kT, c2, False)
                    kq_proj_group(xq_t[qb], wq_sb, qT, c2, True)
                for sc in range(nqc):
                    v_proj_group(sc, qb)

            at_tiles = {}
            v_blocks = {}

            def outproj_block(qb):
                s_lo = qb * QB
                at = at_tiles[qb]
                last = qb == nq - 1
                for qc in range(nqc):
                    o_sb = opool.tile([128, d], BF16, tag="o")
                    for nb in range(d // QB):
                        # in the last block, alternate psum pools (scores pool
                        # is free by then) and evac engines to drain 2x faster
                        if last and nb % 2 == 1:
                            o_ps = sp.tile([128, QB], F32, tag="sp", name="ops")
                        else:
                            o_ps = pp.tile([128, QB], F32, tag="pp", name="ops")
                        for pr in range(npair):
                            nc.tensor.matmul(
                                o_ps,
                                lhsT=at[:, pr, qc * 128 : (qc + 1) * 128],
                                rhs=wo_sb[:, pr, nb * QB : (nb + 1) * QB],
                                start=(pr == 0),
                                stop=(pr == npair - 1),
                            )
                        if last and nb % 2 == 1:
                            nc.scalar.copy(o_sb[:, nb * QB : (nb + 1) * QB], o_ps)
                        else:
                            nc.vector.tensor_copy(
                                o_sb[:, nb * QB : (nb + 1) * QB], o_ps
                            )
                        dma_eng = nc.scalar if (last and nb % 2 == 1) else nc.sync
                        dma_eng.dma_start(
                            out[(s_lo + qc * 128) : (s_lo + (qc + 1) * 128),
                                nb * QB : (nb + 1) * QB],
                            o_sb[:, nb * QB : (nb + 1) * QB],
                        )

            def attn_block(qb):
                s_lo = qb * QB
                n_chunks = (qb + 1) * (QB // KC) if mask_mode == "causal" else nkc
                diag_lo = qb * (QB // KC)

                if mask_mode == "general":
                    m_tiles = []
                    for kc_i in range(n_chunks):
                        mt = mpool.tile([128, QB], BF16, tag="m")
                        nc.sync.dma_start(
                            mt, m01[kc_i * KC : (kc_i + 1) * KC, s_lo : s_lo + QB]
                        )
                        m_tiles.append(mt)

                # A2[(qc, pr)] [128 q, 128] collects heads (2pr, 2pr+1)
                a2 = {}
                for pr in range(npair):
                    for qc in range(nqc):
                        a2[(qc, pr)] = a2pool.tile(
                            [128, 128], BF16, tag="a2", name=f"a2_{qc}_{pr}"
                        )

                assert n_chunks % 2 == 0
                at = atpool.tile([128, npair, QB], BF16, tag="at")
                for pr in range(npair):
                    # Per head: one packed PSUM bank [128, 4, 65] accumulates
                    # all 4 q-chunks (flipped PV).  Only the very first matmul
                    # issues start=True (clears the bank's has_written bits);
                    # other chains' first matmuls overwrite-where-unset.  PV
                    # chunk-matmuls stream right behind each exp so the PE
                    # always has work while ACT processes the next chunk.
                    a_pss = [
                        app.tile([128, nqc, 65], F32, tag=app_tag, name=f"aps{sub}")
                        for sub in range(2)
                    ]
                    started = [False, False]
                    for kcp in range(0, n_chunks, 2):
                        if mask_mode == "causal":
                            skips = [
                                max(0, ((kcp + ck) - diag_lo) * KC)
                                for ck in range(2)
                            ]
                        else:
                            skips = [0, 0]
                        for sub in range(2):
                            h = pr * 2 + sub
                            rows = slice(sub * 64, sub * 64 + 64)
                            s_ps = sp.tile([128, 2 * QB], F32, tag="sp", name="sps")
                            with tc.high_priority():
                                for ck in range(2):
                                    kc_i = kcp + ck
                                    sk = skips[ck]
                                    diag = (mask_mode == "causal" and kc_i >= diag_lo
                                            and not pool_mask)
                                    nc.tensor.matmul(
                                        s_ps[:, ck * QB + sk : (ck + 1) * QB],
                                        lhsT=kT[rows, pr, kc_i * KC : (kc_i + 1) * KC],
                                        rhs=qT[rows, pr, s_lo + sk : s_lo + QB],
                                        start=True,
                                        stop=not diag,
                                    )
                                    if diag:
                                        # additive -1e9 triangle on the diagonal
                                        # 128x128 via identity-lhsT matmul
                                        nc.tensor.matmul(
                                            s_ps[:, ck * QB + sk : ck * QB + sk + 128],
                                            lhsT=ident,
                                            rhs=ctri,
                                            start=False,
                                            stop=True,
                                            skip_group_check=True,
                                        )
                                e = epool.tile([128, 2 * QB], BF16, tag="e")
                                if skips[0] == 0 and skips[1] == 0:
                                    nc.scalar.activation(
                                        out=e, in_=s_ps,
                                        func=mybir.ActivationFunctionType.Exp,
                                    )
                                else:
                                    for ck in range(2):
                                        sk = skips[ck]
                                        nc.scalar.activation(
                                            out=e[:, ck * QB + sk : (ck + 1) * QB],
                                            in_=s_ps[:, ck * QB + sk : (ck + 1) * QB],
                                            func=mybir.ActivationFunctionType.Exp,
                                        )
                            a_ps = a_pss[sub]
                            for ck in range(2):
                                kc_i = kcp + ck
                                if (pool_mask and mask_mode == "causal"
                                        and kc_i >= diag_lo):
                                    sk = skips[ck]
                                    nc.gpsimd.tensor_mul(
                                        e[:, ck * QB + sk : ck * QB + sk + 128],
                                        e[:, ck * QB + sk : ck * QB + sk + 128],
                                        tri01,
                                    )
                                if mask_mode == "general":
                                    nc.vector.tensor_mul(
                                        e[:, ck * QB : (ck + 1) * QB],
                                        e[:, ck * QB : (ck + 1) * QB],
                                        m_tiles[kc_i],
                                    )
                                for qc in range(nqc):
                                    gqc = diag_lo + qc
                                    if mask_mode == "causal" and kc_i > gqc:
                                        continue
                                    first = not started[sub]
                                    nc.tensor.matmul(
                                        a_ps[:, qc, :],
                                        lhsT=e[:, ck * QB + qc * 128 : ck * QB + (qc + 1) * 128],
                                        rhs=v_sb[:, kc_i, h, :],
                                        start=first,
                                        stop=(
                                            kc_i == (gqc if mask_mode == "causal" else n_chunks - 1)
                                        ),
                                        skip_group_check=not first,
                                    )
                                    started[sub] = True

                    # normalize via the ones column: per-partition reciprocal
                    # of the denominator, then one scalar-multiply per q-chunk.
                    # In the last q-block the ACT engine is idle, so its odd
                    # heads' multiplies go there to shorten the tail chain.
                    from contextlib import ExitStack as _ES
                    _hp = tc.high_priority() if qb == nq - 1 else nullcontext()
                    with _hp:
                        for sub in range(2):
                            recip = rpool.tile([128, nqc], F32, tag="r")
                            nc.vector.reciprocal(
                                out=recip, in_=a_pss[sub][:, :, 64]
                            )
                            on_act = sub == 1 and qb == nq - 1
                            for qc in range(nqc):
                                dst = a2[(qc, pr)][:, sub * 64 : (sub + 1) * 64]
                                if on_act:
                                    nc.scalar.mul(
                                        dst, a_pss[sub][:, qc, 0:64],
                                        recip[:, qc : qc + 1],
                                    )
                                else:
                                    nc.vector.tensor_scalar_mul(
                                        dst, a_pss[sub][:, qc, 0:64],
                                        recip[:, qc : qc + 1],
                                    )

                    # A^T for this pair (overlaps the other pair's scores)
                    _hp2 = tc.high_priority() if qb == nq - 1 else nullcontext()
                    with _hp2:
                        for qc in range(nqc):
                            if dma_tr:
                                nc.sync.dma_start_transpose(
                                    at[:, pr, qc * 128 : (qc + 1) * 128],
                                    a2[(qc, pr)][:, :],
                                )
                            else:
                                tr = app.tile([128, 128], BF16, tag=app_tag, name="tr")
                                nc.tensor.transpose(tr, a2[(qc, pr)], ident)
                                nc.vector.tensor_copy(
                                    at[:, pr, qc * 128 : (qc + 1) * 128], tr
                                )

                at_tiles[qb] = at

            if mask_mode == "causal":
                # attention(qb) only reads k/v ranges projected so far.
                # v-projections are emitted AFTER the attention block so the
                # score matmuls (which feed the ACT exp stream) win the PE
                # priority race; deps still place v before the diagonal PV.
                # Out-projections are deferred to the end: pure-PE work that
                # fills exp-gated PE idle during the heavy late blocks.
                for qb in range(nq):
                    proj_block(qb)
                    attn_block(qb)
                for qb in range(nq):
                    outproj_block(qb)
            else:
                # unmasked attention reads ALL k/v: project everything first
                for qb in range(nq):
                    proj_block(qb)
                for qb in range(nq):
                    attn_block(qb)
                for qb in range(nq):
                    outproj_block(qb)

    return nc


# ---------------------------------------------------------------------------
# host side
# ---------------------------------------------------------------------------

_PROG_CACHE = {}
LAST_RESULT = None


def _get_program(mask_mode):
    if mask_mode not in _PROG_CACHE:
        _PROG_CACHE[mask_mode] = build_program(mask_mode)
    return _PROG_CACHE[mask_mode]


def _bf16(a):
    return np.ascontiguousarray(a).astype(BF16_NP)


def _pack_w(wT):
    """[D, m] -> [128, (D//128)*m] with row p holding chunks contiguously."""
    dch_, m = wT.shape[0] // 128, wT.shape[1]
    return np.ascontiguousarray(
        wT.reshape(dch_, 128, m).transpose(1, 0, 2).reshape(128, dch_ * m)
    )


def _pack_w_c2(wT):
    """[D, m] -> [128, (m//128)*(D//128)*128]: w[p, (c2, dc, mm)] =
    wT[dc*128+p, c2*128+mm] so one c2-half is a contiguous run per row."""
    dch_, m = wT.shape[0] // 128, wT.shape[1]
    npair_ = m // 128
    return np.ascontiguousarray(
        wT.reshape(dch_, 128, npair_, 128).transpose(1, 2, 0, 3).reshape(128, -1)
    )


def kernel(query, key_in, value, mask, w_q, b_q, w_k, b_k, w_v, b_v, w_out, b_out):
    from concourse.bass_utils import run_bass_kernel_spmd

    query = np.asarray(query, dtype=np.float32)
    key_in = np.asarray(key_in, dtype=np.float32)
    value = np.asarray(value, dtype=np.float32)
    mask = np.asarray(mask)
    w_q = np.asarray(w_q, dtype=np.float32)
    b_q = np.asarray(b_q, dtype=np.float32)
    w_k = np.asarray(w_k, dtype=np.float32)
    b_k = np.asarray(b_k, dtype=np.float32)
    w_v = np.asarray(w_v, dtype=np.float32)
    b_v = np.asarray(b_v, dtype=np.float32)
    w_out = np.asarray(w_out, dtype=np.float32)
    b_out = np.asarray(b_out, dtype=np.float32)

    scale = 1.0 / np.sqrt(np.float32(D_K))

    if (mask == 1).all():
        mode = "ones"
    elif all(
        np.array_equal(mask[b, 0], np.tril(np.ones((S, S), mask.dtype)))
        for b in range(mask.shape[0])
    ):
        mode = "causal"
    else:
        mode = "general"
    nc = _get_program(mode)

    wqT = _bf16(w_q.T * scale)   # [D, D] scaled
    wkT = _bf16(w_k.T)
    wvT = _bf16(w_v.T)
    woT = _bf16(w_out.T)
    bq_s = (b_q * scale).astype(np.float32)

    # per-batch transposed activations, shared by the 4 cores of a batch
    xqT = [_bf16(query[b].T) for b in range(B)]
    xkT = [_bf16(key_in[b].T) for b in range(B)]
    xvT = [_bf16(value[b].T) for b in range(B)]
    m01T = [_bf16(mask[b, 0].T) for b in range(B)] if mode == "general" else None

    in_maps = []
    for c in range(N_CORES):
        b = c // CORES_PER_BATCH
        hg = c % CORES_PER_BATCH
        hsl = slice(hg * HD, (hg + 1) * HD)
        im = {
            "xq": xqT[b],
            "xk": xkT[b],
            "xv": xvT[b],
            "wq": _pack_w_c2(wqT[:, hsl]),
            "wk": _pack_w_c2(wkT[:, hsl]),
            "wv": _pack_w_c2(wvT[:, hsl]),
            "wo": _pack_w(woT[hsl, :]),
            "bq": np.ascontiguousarray(bq_s[hsl].reshape(HD, 1)),
        }
        if mode == "general":
            im["m01"] = m01T[b]
        in_maps.append(im)

    global LAST_RESULT
    try:
        res = run_bass_kernel_spmd(nc, in_maps, list(range(N_CORES)))
    except Exception:
        # transient NRT_EXEC_UNIT_UNRECOVERABLE wedges have been observed on
        # this fabric; a single retry has always cleared them
        import time as _time

        _time.sleep(3.0)
        res = run_bass_kernel_spmd(nc, in_maps, list(range(N_CORES)))
    LAST_RESULT = res

    b_eff = b_out + w_out @ b_v
    out = np.zeros((B, S, D_MODEL), dtype=np.float32)
    for c in range(N_CORES):
        out[c // CORES_PER_BATCH] += res.results[c]["out"].astype(np.float32)
    out += b_eff[None, None, :]
    return out
